# revision 27
# baseline (speedup 1.0000x reference)
"""Trainium2 Bass kernel for a 3-layer FCL + size-5 sliding-window stack.

Reference computation (fp32):
    h = relu(x @ W1.T)          # [N, 10]
    t = relu(h @ W2.T + b2)     # [N, 5]
    out[n] = concat(t[n-2..n+2])  zero-padded  -> [N, 25]

Strategy (8 cores, data-parallel over rows; per-PARTITION row blocking):
  - Each core owns 25000 output rows.  Within a core, partition p owns the
    196 contiguous rows [196p, 196p+196) and computes t for 200 rows
    (its own 196 plus a 2-row halo each side), so the size-5 window never
    crosses a partition: the window expand is a pure in-partition strided
    read and the output store is per-partition contiguous.
  - x is DMA-loaded with a fused f32->bf16 cast (SWDGE on the Pool queue)
    in 10 fat instructions (13/25*8/8/4 rows per partition each; up to
    32000-byte contiguous reads per partition), instead of many small
    ones - the per-instruction DGE/semaphore overheads (~1us each) are
    what kept the DMA engines <55% occupied in the previous version.
  - Compute runs in 50 chunks of 4 rows/partition (512 rows): 12 PE
    transposes put the 320-dim contraction on partitions (bf16 streams
    1 col/cycle); one big PSUM->SBUF copy per chunk (DVE 2 of 3 chunks,
    ACT 1 of 3); L1 accumulates hT[10,512] fp32 in PSUM; ACT relu casts to
    bf16 into an [11,512] tile whose row 10 is constant 1.0; L2 multiplies
    h back to natural layout with W2T augmented by a b2 row (bias folded
    into the matmul); DVE relu-copies t into a persistent SBUF tile
    t_all[128, 200, 5].
  - The sliding window is ONE 4-dim strided DVE read per chunk (lagged one
    chunk) from t_all into double-buffered 28-row out tiles; out stores
    are 7 DMAs with 2800B contiguous per-partition descriptors.  No DRAM
    bounce for t at all.
  - Global zero-padding of the window (4 tiny row slices at the ends of the
    full output) is patched on the host during unsharding.
"""

import numpy as np

import bass_rust
import concourse.bass as bass
import concourse.mybir as mybir
import concourse.tile as tile
from concourse.tile import add_dep_helper

# ---- problem constants (hardcoded per contract) ----
N = 200000
D = 320
D1 = 10
D2 = 5
W = 5
HALF = W // 2
NCORES = 8
ROWS = N // NCORES          # 25000 output rows per core
PPART = 196                 # owned rows per partition (128*196 = 25088)
TROWS = PPART + 4           # t rows per partition incl. 2-row halo each side
OUTPAD = 128 * PPART        # 25088 out rows per core (>= 25000)
XPAD = 128 * PPART + 4      # 25092 x rows per core shard
NCHUNK = TROWS // 4         # 50 compute chunks of 4 rows/partition
# x-load row-blocks per partition (sum 200): moderate first block, small
# blocks at the tail so the PE does not starve at the end of the stream.
RB = [13, 25, 25, 25, 25, 25, 25, 25, 8, 4]
RSTART = [sum(RB[:i]) for i in range(len(RB))]
XTILE_ROWS = max(RB)
OG = 28                     # out rows per store group (7 groups of 7 expands)
CHUNKS = [(0, 128), (128, 128), (256, 64)]  # d-chunks of 320
F32 = mybir.dt.float32
BF16 = mybir.dt.bfloat16
FP8 = mybir.dt.float8e4
RELU = mybir.ActivationFunctionType.Relu

_NC_CACHE = {}


def _dep(a, b, why):
    add_dep_helper(a.ins, b.ins, reason=why)


def split_multiwaits(nc):
    """Walrus/ISA allows ONE sync-wait per instruction; Tile emits several.

    For every instruction with >1 wait, hoist all but the last wait onto
    fresh NoOps on the same engine immediately before it.  The engine
    stalls at the nops exactly as it would have at the instruction, so
    semantics are unchanged.
    """
    n_split = 0
    for bb in nc.main_func.blocks:
        insts = bb.instructions
        out = []
        changed = False
        for ins in insts:
            si = ins.sync_info
            waits = list(si.on_wait) if si is not None else []
            if len(waits) > 1:
                changed = True
                for w in waits[:-1]:
                    n_split += 1
                    nop = bass_rust.InstNoOp(name=f"wsplit-{n_split}")
                    nop.engine = ins.engine
                    nop.sync_info = bass_rust.SyncInfo(
                        on_wait=[w], on_update=[]
                    )
                    nc.inst_map[nop.name] = nop
                    out.append(nop)
                ins.sync_info = bass_rust.SyncInfo(
                    on_wait=[waits[-1]], on_update=list(si.on_update)
                )
            out.append(ins)
        if changed:
            bb.instructions = out
    return n_split


def _block_of(r):
    for b in range(len(RB) - 1, -1, -1):
        if r >= RSTART[b]:
            return b
    raise AssertionError(r)


def build_nc():
    nc = bass.Bass("TRN2", target_bir_lowering=False, debug=False)

    x_t = nc.dram_tensor("x", [XPAD, D], F32, kind="ExternalInput")
    # host-prepped W1.T zero-padded to [384, 10] and [W2.T; b2] [11, 5]
    w1t_t = nc.dram_tensor("W1T", [3 * 128, D1], F32, kind="ExternalInput")
    w2aug_t = nc.dram_tensor("W2AUG", [D1 + 1, D2], F32, kind="ExternalInput")
    out_t = nc.dram_tensor("out", [OUTPAD, W * D2], F32, kind="ExternalOutput")

    with tile.TileContext(nc) as tc:
        with (
            tc.tile_pool(name="singles", bufs=1) as singles,
            tc.tile_pool(name="xpool", bufs=1) as xpool,
            tc.tile_pool(name="xtpool", bufs=4) as xtpool,
            tc.tile_pool(name="ogpool", bufs=2) as ogpool,
            tc.tile_pool(name="ps_xt", bufs=2, space="PSUM") as ps_xt,
            tc.tile_pool(name="ps_h", bufs=2, space="PSUM") as ps_h,
            tc.tile_pool(name="ps_t", bufs=2, space="PSUM") as ps_t,
        ):
            # ---- x loads first: nothing on the Pool queue may delay them
            x_tiles = [None] * len(RB)

            def issue_load(b):
                nrows = RB[b]
                x_sb = xpool.tile([128, XTILE_ROWS, D], BF16, tag=f"x{b % 6}")
                nc.gpsimd.dma_start(
                    out=x_sb[:, :nrows, :],
                    in_=bass.AP(
                        x_t,
                        RSTART[b] * D,
                        [[PPART * D, 128], [1, nrows * D]],
                    ),
                )
                x_tiles[b] = x_sb

            for b in (0, 1, 2, 3, 4):
                issue_load(b)

            # ---- constants ----
            ident = singles.tile([128, 128], BF16)
            nc.gpsimd.memset(ident, 0.0)
            asel = nc.gpsimd.affine_select(
                out=ident,
                in_=ident,
                compare_op=mybir.AluOpType.not_equal,
                fill=1.0,
                base=0,
                pattern=[[-1, 128]],
                channel_multiplier=1,
            )
            # h tiles: [11, 512]; row 10 is the constant-1 bias row.
            h_sbs = []
            for i in range(2):
                h_sb = singles.tile([D1 + 1, 512], BF16, name=f"h_sb{i}")
                nc.gpsimd.memset(h_sb, 1.0)
                h_sbs.append(h_sb)

            # host-transposed weights: one DMA + one DVE cast each.
            w1t_f32 = singles.tile([128, len(CHUNKS), D1], F32)
            nc.sync.dma_start(
                out=w1t_f32,
                in_=bass.AP(w1t_t, 0, [[D1, 128], [128 * D1, 3], [1, D1]]),
            )
            w1t_sb = singles.tile([128, len(CHUNKS), D1], BF16)
            w1cast = nc.vector.tensor_copy(out=w1t_sb, in_=w1t_f32)
            w2aug_f32 = singles.tile([D1 + 1, D2], F32)
            nc.sync.dma_start(out=w2aug_f32, in_=w2aug_t[:, :])
            w2aug_sb = singles.tile([D1 + 1, D2], BF16)
            w2cast = nc.vector.tensor_copy(out=w2aug_sb, in_=w2aug_f32)

            # PE observes the identity build + weight casts once; transposes
            # and matmuls then only wait on their per-chunk producers.
            nop_id = nc.tensor.nop()
            _dep(nop_id, asel, "PE pre-observe identity")
            nop_w = nc.tensor.nop()
            _dep(nop_w, w1cast, "PE pre-observe W1T cast")
            nop_w2 = nc.tensor.nop()
            _dep(nop_w2, w2cast, "PE pre-observe W2AUG cast")

            # persistent per-partition t (incl halo)
            t_all = singles.tile([128, TROWS, D2], F32)

            # pre-warm the ACT relu table during the first x transfer
            # (h_sb0 row 0 is rewritten by the first real relu before use)
            nc.scalar.activation(h_sbs[0][0:1, 0:1], h_sbs[0][0:1, 0:1], RELU)

            # issue load b when compute first touches block b-5: the DMA
            # outpaces the PE, so descriptor-gen must run well ahead or the
            # stream goes idle waiting for gens.  The 6-deep x pool makes
            # the WAR (on block b-6's readers) already satisfied at that
            # point, so gens never block the Pool queue.
            triggers = {}
            for b in range(5, len(RB)):
                q = (RSTART[b - 5] + 3) // 4
                triggers.setdefault(q, []).append(b)

            og_tiles = {}

            def emit_expand(e):
                """out rows 4e..4e+3: window w of row k reads t_all[k+w]."""
                g, i = divmod(e, 7)
                if i == 0:
                    og_tiles[g] = ogpool.tile(
                        [128, OG, W * D2], F32, tag=f"og{g % 2}",
                        name=f"og_{g}",
                    )
                og = og_tiles[g]
                nc.gpsimd.tensor_copy(
                    out=bass.AP(
                        og.tensor,
                        og.offset + 4 * i * W * D2,
                        [
                            [og.ap[0][0], 128],
                            [W * D2, 4],
                            [D2, W],
                            [1, D2],
                        ],
                    ),
                    in_=bass.AP(
                        t_all.tensor,
                        t_all.offset + 4 * e * D2,
                        [
                            [t_all.ap[0][0], 128],
                            [D2, 4],
                            [D2, W],
                            [1, D2],
                        ],
                    ),
                )
                if i == 6:
                    nc.sync.dma_start(
                        out=bass.AP(
                            out_t,
                            OG * g * W * D2,
                            [[PPART * W * D2, 128], [1, OG * W * D2]],
                        ),
                        in_=og[:, :, :],
                    )

            # ---- main loop, software-pipelined so the in-order PE queue
            # never stalls: iteration i emits transposes(i), L1(i-1),
            # L2(i-2) - every PE instruction's producer ran an iteration
            # earlier, so its wait is pre-satisfied.
            xt_sbs = {}
            for i in range(NCHUNK + 2):
                if i < NCHUNK:
                    for b in triggers.get(i, ()):
                        issue_load(b)

                    xt_ps = ps_xt.tile([128, 3, 512], BF16, tag="xt")
                    xt_sb = xtpool.tile([128, 3, 512], BF16, tag="xts")
                    for jj in range(4):
                        r = 4 * i + jj
                        b = _block_of(r)
                        j = r - RSTART[b]
                        for c, (d0, cw) in enumerate(CHUNKS):
                            nc.tensor.transpose(
                                xt_ps[:cw, c, jj * 128 : (jj + 1) * 128],
                                x_tiles[b][:, j, d0 : d0 + cw],
                                ident,
                            )
                    # one big PSUM->SBUF copy on DVE (957ns in 2x mode;
                    # ACT takes 1.5us and overruns the 1.8us chunk period)
                    nc.vector.tensor_copy(out=xt_sb, in_=xt_ps)
                    xt_sbs[i] = xt_sb

                if 1 <= i <= NCHUNK:
                    q = i - 1
                    h_ps = ps_h.tile([D1, 512], F32, tag="h")
                    order = (2, 0, 1)  # 64-wide chunk first
                    for k, c in enumerate(order):
                        d0, cw = CHUNKS[c]
                        nc.tensor.matmul(
                            h_ps,
                            w1t_sb[:cw, c, :],
                            xt_sbs[q][:cw, c, :],
                            start=(k == 0),
                            stop=(k == len(order) - 1),
                        )
                    h_sb = h_sbs[q % 2]
                    nc.scalar.activation(h_sb[:D1, :], h_ps, RELU)

                if 2 <= i <= NCHUNK + 1:
                    q = i - 2
                    h_sb = h_sbs[q % 2]
                    t_ps = ps_t.tile([128, 4, D2], F32, tag="t")
                    for jj in range(4):
                        nc.tensor.matmul(
                            t_ps[:, jj, :],
                            h_sb[:, jj * 128 : (jj + 1) * 128],
                            w2aug_sb,
                            start=True,
                            stop=True,
                        )
                    # fused bias is already in; relu while copying to t_all
                    nc.vector.tensor_scalar_max(
                        t_all[:, 4 * q : 4 * q + 4, :], t_ps, 0.0
                    )

                if 3 <= i:
                    emit_expand(i - 3)

    split_multiwaits(nc)
    return nc


def make_shards(x):
    """Per-core [XPAD, D] shards; shard row i = global row 25000c + i - 2."""
    shards = []
    for c in range(NCORES):
        s = np.zeros((XPAD, D), dtype=np.float32)
        lo = ROWS * c - HALF
        hi = lo + XPAD
        src_lo, src_hi = max(lo, 0), min(hi, N)
        s[src_lo - lo : src_lo - lo + (src_hi - src_lo)] = x[src_lo:src_hi]
        shards.append(s)
    return shards


def _patch_edges(out):
    # the reference zero-pads t, not x: window slots that fall outside
    # [0, N) must be exactly zero.
    out[0, : 2 * D2] = 0.0
    out[1, :D2] = 0.0
    out[N - 2, 4 * D2 :] = 0.0
    out[N - 1, 3 * D2 :] = 0.0
    return out


def run(inputs, trace=False):
    from concourse.bass_utils import run_bass_kernel_spmd

    x = np.ascontiguousarray(np.asarray(inputs["x"], dtype=np.float32))
    W1 = np.asarray(inputs["W1"], dtype=np.float32)
    W2 = np.asarray(inputs["W2"], dtype=np.float32)
    b2 = np.asarray(inputs["b2"], dtype=np.float32)
    assert x.shape == (N, D)

    # host-prepped weights: W1.T zero-padded to [384, 10]; [W2.T; b2]
    w1t = np.zeros((3 * 128, D1), dtype=np.float32)
    w1t[:D] = W1.T
    w2aug = np.ascontiguousarray(
        np.concatenate([W2.T, b2[None, :]], axis=0), dtype=np.float32
    )

    if "nc" not in _NC_CACHE:
        _NC_CACHE["nc"] = build_nc()
    nc = _NC_CACHE["nc"]

    in_maps = [
        {"x": s, "W1T": w1t, "W2AUG": w2aug} for s in make_shards(x)
    ]
    res = run_bass_kernel_spmd(nc, in_maps, list(range(NCORES)), trace=trace)
    out = np.concatenate(
        [res.results[c]["out"][:ROWS] for c in range(NCORES)], axis=0
    )
    return _patch_edges(out), res


def kernel(**inputs):
    out, _ = run(inputs, trace=False)
    return out


# revision 28
# speedup vs baseline: 1.0124x; 1.0124x over previous
"""Trainium2 Bass kernel for a 3-layer FCL + size-5 sliding-window stack.

Reference computation (fp32):
    h = relu(x @ W1.T)          # [N, 10]
    t = relu(h @ W2.T + b2)     # [N, 5]
    out[n] = concat(t[n-2..n+2])  zero-padded  -> [N, 25]

Strategy (8 cores, data-parallel over rows; per-PARTITION row blocking):
  - Each core owns 25000 output rows.  Within a core, partition p owns the
    196 contiguous rows [196p, 196p+196) and computes t for 200 rows
    (its own 196 plus a 2-row halo each side), so the size-5 window never
    crosses a partition: the window expand is a pure in-partition strided
    read and the output store is per-partition contiguous.
  - x is DMA-loaded with a fused f32->bf16 cast (SWDGE on the Pool queue)
    in 10 fat instructions (13/25*8/8/4 rows per partition each; up to
    32000-byte contiguous reads per partition), instead of many small
    ones - the per-instruction DGE/semaphore overheads (~1us each) are
    what kept the DMA engines <55% occupied in the previous version.
  - Compute runs in 50 chunks of 4 rows/partition (512 rows): 12 PE
    transposes put the 320-dim contraction on partitions (bf16 streams
    1 col/cycle); one big PSUM->SBUF copy per chunk (DVE 2 of 3 chunks,
    ACT 1 of 3); L1 accumulates hT[10,512] fp32 in PSUM; ACT relu casts to
    bf16 into an [11,512] tile whose row 10 is constant 1.0; L2 multiplies
    h back to natural layout with W2T augmented by a b2 row (bias folded
    into the matmul); DVE relu-copies t into a persistent SBUF tile
    t_all[128, 200, 5].
  - The sliding window is ONE 4-dim strided DVE read per chunk (lagged one
    chunk) from t_all into double-buffered 28-row out tiles; out stores
    are 7 DMAs with 2800B contiguous per-partition descriptors.  No DRAM
    bounce for t at all.
  - Global zero-padding of the window (4 tiny row slices at the ends of the
    full output) is patched on the host during unsharding.
"""

import numpy as np

import bass_rust
import concourse.bass as bass
import concourse.mybir as mybir
import concourse.tile as tile
from concourse.tile import add_dep_helper

# ---- problem constants (hardcoded per contract) ----
N = 200000
D = 320
D1 = 10
D2 = 5
W = 5
HALF = W // 2
NCORES = 8
ROWS = N // NCORES          # 25000 output rows per core
PPART = 196                 # owned rows per partition (128*196 = 25088)
TROWS = PPART + 4           # t rows per partition incl. 2-row halo each side
OUTPAD = 128 * PPART        # 25088 out rows per core (>= 25000)
XPAD = 128 * PPART + 4      # 25092 x rows per core shard
NCHUNK = TROWS // 4         # 50 compute chunks of 4 rows/partition
# x-load row-blocks per partition (sum 200): uniform small blocks keep the
# per-block completion-semaphore granularity low, so the PE tracks the
# stream closely instead of stalling a whole 25-row block at a time.
RB = [13] * 15 + [5]
RSTART = [sum(RB[:i]) for i in range(len(RB))]
XTILE_ROWS = max(RB)
OG = 28                     # out rows per store group (7 groups of 7 expands)
CHUNKS = [(0, 128), (128, 128), (256, 64)]  # d-chunks of 320
F32 = mybir.dt.float32
BF16 = mybir.dt.bfloat16
FP8 = mybir.dt.float8e4
RELU = mybir.ActivationFunctionType.Relu

_NC_CACHE = {}


def _dep(a, b, why):
    add_dep_helper(a.ins, b.ins, reason=why)


def split_multiwaits(nc):
    """Walrus/ISA allows ONE sync-wait per instruction; Tile emits several.

    For every instruction with >1 wait, hoist all but the last wait onto
    fresh NoOps on the same engine immediately before it.  The engine
    stalls at the nops exactly as it would have at the instruction, so
    semantics are unchanged.
    """
    n_split = 0
    for bb in nc.main_func.blocks:
        insts = bb.instructions
        out = []
        changed = False
        for ins in insts:
            si = ins.sync_info
            waits = list(si.on_wait) if si is not None else []
            if len(waits) > 1:
                changed = True
                for w in waits[:-1]:
                    n_split += 1
                    nop = bass_rust.InstNoOp(name=f"wsplit-{n_split}")
                    nop.engine = ins.engine
                    nop.sync_info = bass_rust.SyncInfo(
                        on_wait=[w], on_update=[]
                    )
                    nc.inst_map[nop.name] = nop
                    out.append(nop)
                ins.sync_info = bass_rust.SyncInfo(
                    on_wait=[waits[-1]], on_update=list(si.on_update)
                )
            out.append(ins)
        if changed:
            bb.instructions = out
    return n_split


def _block_of(r):
    for b in range(len(RB) - 1, -1, -1):
        if r >= RSTART[b]:
            return b
    raise AssertionError(r)


def build_nc():
    nc = bass.Bass("TRN2", target_bir_lowering=False, debug=False)

    x_t = nc.dram_tensor("x", [XPAD, D], F32, kind="ExternalInput")
    # host-prepped W1.T zero-padded to [384, 10] and [W2.T; b2] [11, 5]
    w1t_t = nc.dram_tensor("W1T", [3 * 128, D1], F32, kind="ExternalInput")
    w2aug_t = nc.dram_tensor("W2AUG", [D1 + 1, D2], F32, kind="ExternalInput")
    out_t = nc.dram_tensor("out", [OUTPAD, W * D2], F32, kind="ExternalOutput")

    with tile.TileContext(nc) as tc:
        with (
            tc.tile_pool(name="singles", bufs=1) as singles,
            tc.tile_pool(name="xpool", bufs=1) as xpool,
            tc.tile_pool(name="xtpool", bufs=4) as xtpool,
            tc.tile_pool(name="ogpool", bufs=2) as ogpool,
            tc.tile_pool(name="ps_xt", bufs=2, space="PSUM") as ps_xt,
            tc.tile_pool(name="ps_h", bufs=2, space="PSUM") as ps_h,
            tc.tile_pool(name="ps_t", bufs=2, space="PSUM") as ps_t,
        ):
            # ---- x loads first: nothing on the Pool queue may delay them
            x_tiles = [None] * len(RB)

            def issue_load(b):
                nrows = RB[b]
                x_sb = xpool.tile([128, XTILE_ROWS, D], BF16, tag=f"x{b % 6}")
                nc.gpsimd.dma_start(
                    out=x_sb[:, :nrows, :],
                    in_=bass.AP(
                        x_t,
                        RSTART[b] * D,
                        [[PPART * D, 128], [1, nrows * D]],
                    ),
                )
                x_tiles[b] = x_sb

            for b in (0, 1, 2, 3, 4):
                issue_load(b)

            # ---- constants ----
            ident = singles.tile([128, 128], BF16)
            nc.gpsimd.memset(ident, 0.0)
            asel = nc.gpsimd.affine_select(
                out=ident,
                in_=ident,
                compare_op=mybir.AluOpType.not_equal,
                fill=1.0,
                base=0,
                pattern=[[-1, 128]],
                channel_multiplier=1,
            )
            # h tiles: [11, 512]; row 10 is the constant-1 bias row.
            h_sbs = []
            for i in range(2):
                h_sb = singles.tile([D1 + 1, 512], BF16, name=f"h_sb{i}")
                nc.gpsimd.memset(h_sb, 1.0)
                h_sbs.append(h_sb)

            # host-transposed weights: one DMA + one DVE cast each.
            w1t_f32 = singles.tile([128, len(CHUNKS), D1], F32)
            nc.sync.dma_start(
                out=w1t_f32,
                in_=bass.AP(w1t_t, 0, [[D1, 128], [128 * D1, 3], [1, D1]]),
            )
            w1t_sb = singles.tile([128, len(CHUNKS), D1], BF16)
            w1cast = nc.vector.tensor_copy(out=w1t_sb, in_=w1t_f32)
            w2aug_f32 = singles.tile([D1 + 1, D2], F32)
            nc.sync.dma_start(out=w2aug_f32, in_=w2aug_t[:, :])
            w2aug_sb = singles.tile([D1 + 1, D2], BF16)
            w2cast = nc.vector.tensor_copy(out=w2aug_sb, in_=w2aug_f32)

            # PE observes the identity build + weight casts once; transposes
            # and matmuls then only wait on their per-chunk producers.
            nop_id = nc.tensor.nop()
            _dep(nop_id, asel, "PE pre-observe identity")
            nop_w = nc.tensor.nop()
            _dep(nop_w, w1cast, "PE pre-observe W1T cast")
            nop_w2 = nc.tensor.nop()
            _dep(nop_w2, w2cast, "PE pre-observe W2AUG cast")

            # persistent per-partition t (incl halo)
            t_all = singles.tile([128, TROWS, D2], F32)

            # pre-warm the ACT relu table during the first x transfer
            # (h_sb0 row 0 is rewritten by the first real relu before use)
            nc.scalar.activation(h_sbs[0][0:1, 0:1], h_sbs[0][0:1, 0:1], RELU)

            # issue load b when compute first touches block b-5: the DMA
            # outpaces the PE, so descriptor-gen must run well ahead or the
            # stream goes idle waiting for gens.  The 6-deep x pool makes
            # the WAR (on block b-6's readers) already satisfied at that
            # point, so gens never block the Pool queue.
            triggers = {}
            for b in range(5, len(RB)):
                q = (RSTART[b - 5] + 3) // 4
                triggers.setdefault(q, []).append(b)

            og_tiles = {}

            def emit_expand(e):
                """out rows 4e..4e+3: window w of row k reads t_all[k+w]."""
                g, i = divmod(e, 7)
                if i == 0:
                    og_tiles[g] = ogpool.tile(
                        [128, OG, W * D2], F32, tag=f"og{g % 2}",
                        name=f"og_{g}",
                    )
                og = og_tiles[g]
                nc.gpsimd.tensor_copy(
                    out=bass.AP(
                        og.tensor,
                        og.offset + 4 * i * W * D2,
                        [
                            [og.ap[0][0], 128],
                            [W * D2, 4],
                            [D2, W],
                            [1, D2],
                        ],
                    ),
                    in_=bass.AP(
                        t_all.tensor,
                        t_all.offset + 4 * e * D2,
                        [
                            [t_all.ap[0][0], 128],
                            [D2, 4],
                            [D2, W],
                            [1, D2],
                        ],
                    ),
                )
                if i == 6:
                    nc.sync.dma_start(
                        out=bass.AP(
                            out_t,
                            OG * g * W * D2,
                            [[PPART * W * D2, 128], [1, OG * W * D2]],
                        ),
                        in_=og[:, :, :],
                    )

            # ---- main loop, software-pipelined so the in-order PE queue
            # never stalls: iteration i emits transposes(i), L1(i-1),
            # L2(i-2) - every PE instruction's producer ran an iteration
            # earlier, so its wait is pre-satisfied.
            xt_sbs = {}
            for i in range(NCHUNK + 2):
                if i < NCHUNK:
                    for b in triggers.get(i, ()):
                        issue_load(b)

                    xt_ps = ps_xt.tile([128, 3, 512], BF16, tag="xt")
                    xt_sb = xtpool.tile([128, 3, 512], BF16, tag="xts")
                    for jj in range(4):
                        r = 4 * i + jj
                        b = _block_of(r)
                        j = r - RSTART[b]
                        for c, (d0, cw) in enumerate(CHUNKS):
                            nc.tensor.transpose(
                                xt_ps[:cw, c, jj * 128 : (jj + 1) * 128],
                                x_tiles[b][:, j, d0 : d0 + cw],
                                ident,
                            )
                    # one big PSUM->SBUF copy on DVE (957ns in 2x mode;
                    # ACT takes 1.5us and overruns the 1.8us chunk period)
                    nc.vector.tensor_copy(out=xt_sb, in_=xt_ps)
                    xt_sbs[i] = xt_sb

                if 1 <= i <= NCHUNK:
                    q = i - 1
                    h_ps = ps_h.tile([D1, 512], F32, tag="h")
                    order = (2, 0, 1)  # 64-wide chunk first
                    for k, c in enumerate(order):
                        d0, cw = CHUNKS[c]
                        nc.tensor.matmul(
                            h_ps,
                            w1t_sb[:cw, c, :],
                            xt_sbs[q][:cw, c, :],
                            start=(k == 0),
                            stop=(k == len(order) - 1),
                        )
                    h_sb = h_sbs[q % 2]
                    nc.scalar.activation(h_sb[:D1, :], h_ps, RELU)

                if 2 <= i <= NCHUNK + 1:
                    q = i - 2
                    h_sb = h_sbs[q % 2]
                    t_ps = ps_t.tile([128, 4, D2], F32, tag="t")
                    for jj in range(4):
                        nc.tensor.matmul(
                            t_ps[:, jj, :],
                            h_sb[:, jj * 128 : (jj + 1) * 128],
                            w2aug_sb,
                            start=True,
                            stop=True,
                        )
                    # fused bias is already in; relu while copying to t_all
                    nc.vector.tensor_scalar_max(
                        t_all[:, 4 * q : 4 * q + 4, :], t_ps, 0.0
                    )

                if 3 <= i:
                    emit_expand(i - 3)

    split_multiwaits(nc)
    return nc


def make_shards(x):
    """Per-core [XPAD, D] shards; shard row i = global row 25000c + i - 2."""
    shards = []
    for c in range(NCORES):
        s = np.zeros((XPAD, D), dtype=np.float32)
        lo = ROWS * c - HALF
        hi = lo + XPAD
        src_lo, src_hi = max(lo, 0), min(hi, N)
        s[src_lo - lo : src_lo - lo + (src_hi - src_lo)] = x[src_lo:src_hi]
        shards.append(s)
    return shards


def _patch_edges(out):
    # the reference zero-pads t, not x: window slots that fall outside
    # [0, N) must be exactly zero.
    out[0, : 2 * D2] = 0.0
    out[1, :D2] = 0.0
    out[N - 2, 4 * D2 :] = 0.0
    out[N - 1, 3 * D2 :] = 0.0
    return out


def run(inputs, trace=False):
    from concourse.bass_utils import run_bass_kernel_spmd

    x = np.ascontiguousarray(np.asarray(inputs["x"], dtype=np.float32))
    W1 = np.asarray(inputs["W1"], dtype=np.float32)
    W2 = np.asarray(inputs["W2"], dtype=np.float32)
    b2 = np.asarray(inputs["b2"], dtype=np.float32)
    assert x.shape == (N, D)

    # host-prepped weights: W1.T zero-padded to [384, 10]; [W2.T; b2]
    w1t = np.zeros((3 * 128, D1), dtype=np.float32)
    w1t[:D] = W1.T
    w2aug = np.ascontiguousarray(
        np.concatenate([W2.T, b2[None, :]], axis=0), dtype=np.float32
    )

    if "nc" not in _NC_CACHE:
        _NC_CACHE["nc"] = build_nc()
    nc = _NC_CACHE["nc"]

    in_maps = [
        {"x": s, "W1T": w1t, "W2AUG": w2aug} for s in make_shards(x)
    ]
    res = run_bass_kernel_spmd(nc, in_maps, list(range(NCORES)), trace=trace)
    out = np.concatenate(
        [res.results[c]["out"][:ROWS] for c in range(NCORES)], axis=0
    )
    return _patch_edges(out), res


def kernel(**inputs):
    out, _ = run(inputs, trace=False)
    return out


# revision 36
# speedup vs baseline: 1.0803x; 1.0671x over previous
"""Trainium2 Bass kernel for a 3-layer FCL + size-5 sliding-window stack.

Reference computation (fp32):
    h = relu(x @ W1.T)          # [N, 10]
    t = relu(h @ W2.T + b2)     # [N, 5]
    out[n] = concat(t[n-2..n+2])  zero-padded  -> [N, 25]

Strategy (8 cores, data-parallel over rows; per-PARTITION row blocking):
  - Each core owns 25000 output rows.  Within a core, partition p owns the
    196 contiguous rows [196p, 196p+196) and computes t for 200 rows
    (its own 196 plus a 2-row halo each side), so the size-5 window never
    crosses a partition: the window expand is a pure in-partition strided
    read and the output store is per-partition contiguous.
  - x is DMA-loaded with a fused f32->bf16 cast (SWDGE on the Pool queue)
    in 10 fat instructions (13/25*8/8/4 rows per partition each; up to
    32000-byte contiguous reads per partition), instead of many small
    ones - the per-instruction DGE/semaphore overheads (~1us each) are
    what kept the DMA engines <55% occupied in the previous version.
  - Compute runs in 50 chunks of 4 rows/partition (512 rows): 12 PE
    transposes put the 320-dim contraction on partitions (bf16 streams
    1 col/cycle); one big PSUM->SBUF copy per chunk (DVE 2 of 3 chunks,
    ACT 1 of 3); L1 accumulates hT[10,512] fp32 in PSUM; ACT relu casts to
    bf16 into an [11,512] tile whose row 10 is constant 1.0; L2 multiplies
    h back to natural layout with W2T augmented by a b2 row (bias folded
    into the matmul); DVE relu-copies t into a persistent SBUF tile
    t_all[128, 200, 5].
  - The sliding window is ONE 4-dim strided DVE read per chunk (lagged one
    chunk) from t_all into double-buffered 28-row out tiles; out stores
    are 7 DMAs with 2800B contiguous per-partition descriptors.  No DRAM
    bounce for t at all.
  - Global zero-padding of the window (4 tiny row slices at the ends of the
    full output) is patched on the host during unsharding.
"""

import numpy as np

import bass_rust
import concourse.bass as bass
import concourse.mybir as mybir
import concourse.tile as tile
from concourse.tile import add_dep_helper

# ---- problem constants (hardcoded per contract) ----
N = 200000
D = 320
D1 = 10
D2 = 5
W = 5
HALF = W // 2
NCORES = 8
ROWS = N // NCORES          # 25000 output rows per core
PPART = 196                 # owned rows per partition (128*196 = 25088)
TROWS = PPART + 4           # t rows per partition incl. 2-row halo each side
OUTPAD = 128 * PPART        # 25088 out rows per core (>= 25000)
XPAD = 128 * PPART + 4      # 25092 x rows per core shard
NCHUNK = TROWS // 4         # 50 compute chunks of 4 rows/partition
# x-load row-blocks per partition (sum 200): uniform small blocks keep the
# per-block completion-semaphore granularity low, so the PE tracks the
# stream closely instead of stalling a whole 25-row block at a time.
# Even sizes so row-PAIRS (packed c2 transposes) never straddle blocks.
RB = [12] * 16 + [8]
RSTART = [sum(RB[:i]) for i in range(len(RB))]
XTILE_ROWS = max(RB)
OG = 28                     # out rows per store group (7 groups of 7 expands)
CHUNKS = [(0, 128), (128, 128), (256, 64)]  # d-chunks of 320
F32 = mybir.dt.float32
BF16 = mybir.dt.bfloat16
FP8 = mybir.dt.float8e4
RELU = mybir.ActivationFunctionType.Relu

_NC_CACHE = {}


def _dep(a, b, why):
    add_dep_helper(a.ins, b.ins, reason=why)


def split_multiwaits(nc):
    """Walrus/ISA allows ONE sync-wait per instruction; Tile emits several.

    For every instruction with >1 wait, hoist all but the last wait onto
    fresh NoOps on the same engine immediately before it.  The engine
    stalls at the nops exactly as it would have at the instruction, so
    semantics are unchanged.
    """
    n_split = 0
    for bb in nc.main_func.blocks:
        insts = bb.instructions
        out = []
        changed = False
        for ins in insts:
            si = ins.sync_info
            waits = list(si.on_wait) if si is not None else []
            if len(waits) > 1:
                changed = True
                for w in waits[:-1]:
                    n_split += 1
                    nop = bass_rust.InstNoOp(name=f"wsplit-{n_split}")
                    nop.engine = ins.engine
                    nop.sync_info = bass_rust.SyncInfo(
                        on_wait=[w], on_update=[]
                    )
                    nc.inst_map[nop.name] = nop
                    out.append(nop)
                ins.sync_info = bass_rust.SyncInfo(
                    on_wait=[waits[-1]], on_update=list(si.on_update)
                )
            out.append(ins)
        if changed:
            bb.instructions = out
    return n_split


def _block_of(r):
    for b in range(len(RB) - 1, -1, -1):
        if r >= RSTART[b]:
            return b
    raise AssertionError(r)


def build_nc():
    nc = bass.Bass("TRN2", target_bir_lowering=False, debug=False)

    x_t = nc.dram_tensor("x", [XPAD, D], F32, kind="ExternalInput")
    # host-prepped W1.T as 4 chunks of 128 rows: [0:128], [128:256],
    # [256:320]+zeros, and zeros+[256:320] again at partitions 64-127 (the
    # packed c2 transpose puts row-pair halves on partition halves).
    w1t_t = nc.dram_tensor("W1T", [4 * 128, D1], F32, kind="ExternalInput")
    w2aug_t = nc.dram_tensor("W2AUG", [D1 + 1, D2], F32, kind="ExternalInput")
    out_t = nc.dram_tensor("out", [OUTPAD, W * D2], F32, kind="ExternalOutput")

    with tile.TileContext(nc) as tc:
        with (
            tc.tile_pool(name="singles", bufs=1) as singles,
            tc.tile_pool(name="xpool", bufs=1) as xpool,
            tc.tile_pool(name="xtpool", bufs=4) as xtpool,
            tc.tile_pool(name="ogpool", bufs=2) as ogpool,
            tc.tile_pool(name="ps_xt", bufs=2, space="PSUM") as ps_xt,
            tc.tile_pool(name="ps_h", bufs=2, space="PSUM") as ps_h,
            tc.tile_pool(name="ps_t", bufs=2, space="PSUM") as ps_t,
        ):
            # ---- x loads first: nothing on the Pool queue may delay them
            x_tiles = [None] * len(RB)

            def issue_load(b):
                nrows = RB[b]
                x_sb = xpool.tile([128, XTILE_ROWS, D], BF16, tag=f"x{b % 6}")
                nc.gpsimd.dma_start(
                    out=x_sb[:, :nrows, :],
                    in_=bass.AP(
                        x_t,
                        RSTART[b] * D,
                        [[PPART * D, 128], [1, nrows * D]],
                    ),
                )
                x_tiles[b] = x_sb

            for b in (0, 1, 2, 3, 4):
                issue_load(b)

            # ---- constants ----
            ident = singles.tile([128, 128], BF16)
            nc.gpsimd.memset(ident, 0.0)
            asel = nc.gpsimd.affine_select(
                out=ident,
                in_=ident,
                compare_op=mybir.AluOpType.not_equal,
                fill=1.0,
                base=0,
                pattern=[[-1, 128]],
                channel_multiplier=1,
            )
            # h tiles: [11, 512]; row 10 is the constant-1 bias row.
            h_sbs = []
            for i in range(2):
                h_sb = singles.tile([D1 + 1, 512], BF16, name=f"h_sb{i}")
                nc.gpsimd.memset(h_sb, 1.0)
                h_sbs.append(h_sb)

            # host-transposed weights: one DMA + one DVE cast each.
            w1t_f32 = singles.tile([128, 4, D1], F32)
            nc.sync.dma_start(
                out=w1t_f32,
                in_=bass.AP(w1t_t, 0, [[D1, 128], [128 * D1, 4], [1, D1]]),
            )
            w1t_sb = singles.tile([128, 4, D1], BF16)
            w1cast = nc.vector.tensor_copy(out=w1t_sb, in_=w1t_f32)
            w2aug_f32 = singles.tile([D1 + 1, D2], F32)
            nc.sync.dma_start(out=w2aug_f32, in_=w2aug_t[:, :])
            w2aug_sb = singles.tile([D1 + 1, D2], BF16)
            w2cast = nc.vector.tensor_copy(out=w2aug_sb, in_=w2aug_f32)

            # PE observes the identity build + weight casts once; transposes
            # and matmuls then only wait on their per-chunk producers.
            nop_id = nc.tensor.nop()
            _dep(nop_id, asel, "PE pre-observe identity")
            nop_w = nc.tensor.nop()
            _dep(nop_w, w1cast, "PE pre-observe W1T cast")
            nop_w2 = nc.tensor.nop()
            _dep(nop_w2, w2cast, "PE pre-observe W2AUG cast")

            # persistent per-partition t (incl halo)
            t_all = singles.tile([128, TROWS, D2], F32)

            # pre-warm the ACT relu table during the first x transfer
            # (h_sb0 row 0 is rewritten by the first real relu before use)
            nc.scalar.activation(h_sbs[0][0:1, 0:1], h_sbs[0][0:1, 0:1], RELU)

            # issue load b when compute first touches block b-5: the DMA
            # outpaces the PE, so descriptor-gen must run well ahead or the
            # stream goes idle waiting for gens.  The 6-deep x pool makes
            # the WAR (on block b-6's readers) already satisfied at that
            # point, so gens never block the Pool queue.
            triggers = {}
            for b in range(5, len(RB)):
                q = (RSTART[b - 5] + 3) // 4
                triggers.setdefault(q, []).append(b)

            og_tiles = {}

            def emit_expand(e):
                """out rows 4e..4e+3: window w of row k reads t_all[k+w]."""
                g, i = divmod(e, 7)
                if i == 0:
                    og_tiles[g] = ogpool.tile(
                        [128, OG, W * D2], F32, tag=f"og{g % 2}",
                        name=f"og_{g}",
                    )
                og = og_tiles[g]
                nc.gpsimd.tensor_copy(
                    out=bass.AP(
                        og.tensor,
                        og.offset + 4 * i * W * D2,
                        [
                            [og.ap[0][0], 128],
                            [W * D2, 4],
                            [D2, W],
                            [1, D2],
                        ],
                    ),
                    in_=bass.AP(
                        t_all.tensor,
                        t_all.offset + 4 * e * D2,
                        [
                            [t_all.ap[0][0], 128],
                            [D2, 4],
                            [D2, W],
                            [1, D2],
                        ],
                    ),
                )
                if i == 6:
                    nc.sync.dma_start(
                        out=bass.AP(
                            out_t,
                            OG * g * W * D2,
                            [[PPART * W * D2, 128], [1, OG * W * D2]],
                        ),
                        in_=og[:, :, :],
                    )

            # ---- main loop, software-pipelined so the in-order PE queue
            # never stalls: iteration i emits transposes(i), L1(i-1),
            # L2(i-2) - every PE instruction's producer ran an iteration
            # earlier, so its wait is pre-satisfied.
            xt_sbs = {}
            for i in range(NCHUNK + 2):
                if i < NCHUNK:
                    for b in triggers.get(i, ()):
                        issue_load(b)

                    xt_ps = ps_xt.tile([128, 3, 512], BF16, tag="xt")
                    xt_sb = xtpool.tile([128, 3, 512], BF16, tag="xts")
                    for jj in range(4):
                        r = 4 * i + jj
                        b = _block_of(r)
                        j = r - RSTART[b]
                        for c, (d0, cw) in enumerate(CHUNKS):
                            nc.tensor.transpose(
                                xt_ps[:cw, c, jj * 128 : (jj + 1) * 128],
                                x_tiles[b][:, j, d0 : d0 + cw],
                                ident,
                            )
                    # PSUM->SBUF copy on DVE (957ns in 2x mode)
                    nc.vector.tensor_copy(out=xt_sb, in_=xt_ps)
                    xt_sbs[i] = xt_sb

                if 2 <= i <= NCHUNK + 1:
                    q = i - 2
                    h_sb = h_sbs[q % 2]
                    t_ps = ps_t.tile([128, 4, D2], F32, tag="t")
                    for jj in range(4):
                        nc.tensor.matmul(
                            t_ps[:, jj, :],
                            h_sb[:, jj * 128 : (jj + 1) * 128],
                            w2aug_sb,
                            start=True,
                            stop=True,
                        )
                    # fused bias is already in; relu while copying to t_all
                    nc.vector.tensor_scalar_max(
                        t_all[:, 4 * q : 4 * q + 4, :], t_ps, 0.0
                    )

                if 1 <= i <= NCHUNK:
                    q = i - 1
                    h_ps = ps_h.tile([D1, 512], F32, tag="h")
                    order = (2, 0, 1)  # 64-wide chunk first
                    for k, c in enumerate(order):
                        d0, cw = CHUNKS[c]
                        nc.tensor.matmul(
                            h_ps,
                            w1t_sb[:cw, c, :],
                            xt_sbs[q][:cw, c, :],
                            start=(k == 0),
                            stop=(k == len(order) - 1),
                        )
                    h_sb = h_sbs[q % 2]
                    nc.scalar.activation(h_sb[:D1, :], h_ps, RELU)

                if 3 <= i:
                    emit_expand(i - 3)

    split_multiwaits(nc)
    return nc


def make_shards(x):
    """Per-core [XPAD, D] shards; shard row i = global row 25000c + i - 2."""
    shards = []
    for c in range(NCORES):
        s = np.zeros((XPAD, D), dtype=np.float32)
        lo = ROWS * c - HALF
        hi = lo + XPAD
        src_lo, src_hi = max(lo, 0), min(hi, N)
        s[src_lo - lo : src_lo - lo + (src_hi - src_lo)] = x[src_lo:src_hi]
        shards.append(s)
    return shards


def _patch_edges(out):
    # the reference zero-pads t, not x: window slots that fall outside
    # [0, N) must be exactly zero.
    out[0, : 2 * D2] = 0.0
    out[1, :D2] = 0.0
    out[N - 2, 4 * D2 :] = 0.0
    out[N - 1, 3 * D2 :] = 0.0
    return out


def run(inputs, trace=False):
    from concourse.bass_utils import run_bass_kernel_spmd

    x = np.ascontiguousarray(np.asarray(inputs["x"], dtype=np.float32))
    W1 = np.asarray(inputs["W1"], dtype=np.float32)
    W2 = np.asarray(inputs["W2"], dtype=np.float32)
    b2 = np.asarray(inputs["b2"], dtype=np.float32)
    assert x.shape == (N, D)

    # host-prepped weights: W1.T in 4 chunks of 128 rows - [0:128],
    # [128:256], [256:320]+pad, and pad+[256:320] (the packed c2 transpose
    # needs the 64-wide tail chunk replicated at partitions 64-127)
    w1t = np.zeros((4 * 128, D1), dtype=np.float32)
    w1t[:D] = W1.T
    w1t[448:512] = W1.T[256:320]
    w2aug = np.ascontiguousarray(
        np.concatenate([W2.T, b2[None, :]], axis=0), dtype=np.float32
    )

    if "nc" not in _NC_CACHE:
        _NC_CACHE["nc"] = build_nc()
    nc = _NC_CACHE["nc"]

    in_maps = [
        {"x": s, "W1T": w1t, "W2AUG": w2aug} for s in make_shards(x)
    ]
    res = run_bass_kernel_spmd(nc, in_maps, list(range(NCORES)), trace=trace)
    out = np.concatenate(
        [res.results[c]["out"][:ROWS] for c in range(NCORES)], axis=0
    )
    return _patch_edges(out), res


def kernel(**inputs):
    out, _ = run(inputs, trace=False)
    return out
